# revision 18
# baseline (speedup 1.0000x reference)
"""Trainium2 Bass kernel for the LeNet C3 dense-conv layer.

Computes out = conv2d_valid(x, K, stride 1) + bias where K is the dense
[16, 6, 5, 5] kernel scattered from the sparse per-branch weights
(w3/w4/w6), x is [128, 6, 256, 256] f32, out is [128, 16, 252, 252] f32.

Strategy:
  - Pure data parallelism: 16 images per NeuronCore across 8 cores.
  - The conv is computed as shift-accumulated banded matmuls into PSUM.
    A block covers 6 output rows of an image pair: the contraction dim
    stacks TWO copies of the 10 input rows (60 partitions each), the
    second copy shifted one column, so each matmul covers two kernel
    columns kx at once: 3 matmuls per block (kx {0,1}, {2,3}, {4})
    instead of 5. lhsT is a host-built banded matrix [120, 96] whose
    column m = c_out*6 + r holds K_dense[c_out, c_in, i-r, kx] at row
    i*6 + c_in (+60 for the kx+1 band). Two images ride in the moving
    free dim (N = 2*256 = 512 = one PSUM bank).
  - fp16 operands (~3e-4 rel err; fp32 accumulation in PSUM). The PE in
    this environment streams 1 column/cycle at 1.2 GHz regardless of
    dtype, so fewer matmul columns == faster.
  - Host pre-packs x into x3[pr, (h,c), j*256+w] (fp16) so each block's
    input is a 2D DMA with >=16 outer units (striped over all 16 SDMA
    engines); the device writes o2[pr, c, h, j*252+w] (f32) with
    2016-byte descriptors, and the host un-interleaves afterwards.
  - PSUM eviction on the vector engine fuses the per-partition bias add.
    The 4 leading pad columns of each input tile keep the 512-wide
    moving operand in-bounds; pads feed only discarded PSUM columns
    (image 1's "pad" is image 0's tail, same deal).
"""

import numpy as np

# LeNet-5 C3 sparse channel connectivity (from the model definition).
CH3 = np.array([[0, 1, 2], [1, 2, 3], [2, 3, 4], [3, 4, 5], [0, 4, 5], [0, 1, 5]])
CH4 = np.array([[0, 1, 2, 3], [1, 2, 3, 4], [2, 3, 4, 5], [0, 3, 4, 5],
                [0, 1, 4, 5], [0, 1, 2, 5], [0, 1, 3, 4], [1, 2, 4, 5],
                [0, 2, 3, 5]])

B, C, H, W = 128, 6, 256, 256
CO, HO, WO = 16, 252, 252
NCORES = 8
BPC = B // NCORES           # images per core
NPAIR = BPC // 2            # image pairs per core
KH = KW = 5

R = 6                       # output rows per block
HI = R + 4                  # input rows per block
NBLK = HO // R              # 42 blocks per image pair
KK = C * HI                 # contraction rows per kx copy (60)
MM = CO * R                 # psum partitions (96)

_STATE = None  # cached Bass module so repeat kernel() calls skip re-tracing


def _dense_kernel(w3, w4, w6):
    k = np.zeros((CO, C, KH, KW), np.float32)
    k[np.arange(6)[:, None], CH3] = w3
    k[6 + np.arange(9)[:, None], CH4] = w4
    k[15] = w6[0]
    return k


def _band(kd, kx):
    """Banded lhsT [KK, MM] for kernel column kx: row i*6 + c_in,
    column c_out*R + r, value kd[c_out, c_in, i-r, kx]."""
    out = np.zeros((KK, MM), np.float32)
    for ci in range(C):
        for i in range(HI):
            for r in range(R):
                ky = i - r
                if 0 <= ky < KH:
                    out[i * C + ci, np.arange(CO) * R + r] = kd[:, ci, ky, kx]
    return out


def _build_module():
    import concourse.bacc as bacc
    import concourse.mybir as mybir
    from concourse.tile import TileContext

    f32 = mybir.dt.float32
    f16 = mybir.dt.float16

    # Bacc (not Bass): its compile() runs generate_event_semaphores(),
    # which splits multi-wait instructions to satisfy the TRN2 1-wait-
    # per-instruction constraint walrus enforces.
    nc = bacc.Bacc(None)
    # x3[pr, (h, c), j*256 + w] = x[2*pr + j, c, h, w]  (host pre-pack).
    x_d = nc.dram_tensor("x", [NPAIR, H * C, 2 * W], f16, kind="ExternalInput")
    # w12: [120, 2*96] = [B(0); B(1)] then [B(2); B(3)].  w4x: [60, 96] = B(4).
    w12_d = nc.dram_tensor("w12", [2 * KK, 2 * MM], f16, kind="ExternalInput")
    w4x_d = nc.dram_tensor("w4x", [KK, MM], f16, kind="ExternalInput")
    b1_d = nc.dram_tensor("b1", [MM, 1], f32, kind="ExternalInput")
    # o2[pr, c, h, j*252 + w] = out[2*pr + j, c, h, w]  (host un-packs)
    o_d = nc.dram_tensor("o", [NPAIR, CO, HO, 2 * WO], f32, kind="ExternalOutput")

    with TileContext(nc) as tc:
        with (
            tc.tile_pool(name="wpool", bufs=1) as wp,
            tc.tile_pool(name="inpool", bufs=8) as ip,
            tc.tile_pool(name="outpool", bufs=8) as op,
            tc.tile_pool(name="pspool", bufs=6, space="PSUM") as pp,
        ):
            w12_t = wp.tile([2 * KK, 2 * MM], f16)
            nc.sync.dma_start(w12_t[:], w12_d[:])
            w4x_t = wp.tile([KK, MM], f16)
            nc.sync.dma_start(w4x_t[:], w4x_d[:])
            b1_t = wp.tile([MM, 1], f32)
            nc.sync.dma_start(b1_t[:], b1_d[:])

            # Prime each constant tile on its consuming engine class so
            # steady-state instructions carry few semaphore waits.
            prime_ps = pp.tile([MM, 192], f32, tag="ps")
            nc.tensor.matmul(prime_ps[:], w12_t[:, 0:MM], w12_t[:, 0:192],
                             start=True, stop=True)
            prime_ps2 = pp.tile([MM, 96], f32, tag="ps")
            nc.tensor.matmul(prime_ps2[:], w4x_t[:, 0:MM], w4x_t[:, 0:96],
                             start=True, stop=True)
            prime_b = op.tile([MM, 1], f32, tag="out")
            nc.vector.tensor_scalar_add(prime_b[:], b1_t[:], 0.0)

            for pr in range(NPAIR):
                for g in range(NBLK):
                    h0 = R * g

                    # Input tile: rows 0..59 hold x cols at t-4 (copy 1),
                    # rows 60..119 the same data at t-3 (copy 2, i.e.
                    # pre-shifted one column for the kx+1 band).
                    it = ip.tile([2 * KK, 4 + 2 * W], f16, tag="in")
                    src = x_d[pr, h0 * C:(h0 + HI) * C, :]
                    nc.sync.dma_start(it[0:KK, 4:4 + 2 * W], src)
                    nc.sync.dma_start(it[KK:2 * KK, 3:3 + 2 * W], src)

                    ps = pp.tile([MM, 2, 256], f32, tag="ps")
                    # q=0 -> kx {0,1}; q=2 -> kx {2,3}; q=4 -> kx 4.
                    nc.tensor.matmul(ps[:, :, :], w12_t[:, 0:MM],
                                     it[:, 0:512], start=True, stop=False)
                    nc.tensor.matmul(ps[:, :, :], w12_t[:, MM:2 * MM],
                                     it[:, 2:514], start=False, stop=False)
                    nc.tensor.matmul(ps[:, :, :], w4x_t[:],
                                     it[0:KK, 4:516], start=False, stop=True)

                    ot = op.tile([MM, 2 * WO], f32, tag="out")
                    nc.vector.tensor_scalar_add(
                        ot[:].rearrange("p (j w) -> p j w", j=2),
                        ps[:, :, 4:4 + WO],
                        b1_t[:, 0:1],
                    )
                    nc.scalar.dma_start(o_d[pr, :, h0:h0 + R, :], ot[:])
    nc.compile()
    return nc


def _get_module():
    global _STATE
    if _STATE is None:
        _STATE = _build_module()
    return _STATE


def kernel(x, w3, b3, w4, b4, w6, b6):
    from concourse.bass_utils import run_bass_kernel_spmd

    x = np.asarray(x, np.float32)
    kd = _dense_kernel(np.asarray(w3, np.float32), np.asarray(w4, np.float32),
                       np.asarray(w6, np.float32))
    bias = np.concatenate([np.asarray(b3, np.float32),
                           np.asarray(b4, np.float32),
                           np.asarray(b6, np.float32)])

    w12 = np.concatenate([
        np.concatenate([_band(kd, 0), _band(kd, 2)], axis=1),   # rows 0..59
        np.concatenate([_band(kd, 1), _band(kd, 3)], axis=1),   # rows 60..119
    ], axis=0).astype(np.float16)
    w4x = _band(kd, 4).astype(np.float16)
    b1 = np.repeat(bias, R).astype(np.float32).reshape(MM, 1)

    nc = _get_module()
    x16 = x.astype(np.float16)
    in_maps = []
    for cr in range(NCORES):
        xs = x16[cr * BPC:(cr + 1) * BPC]
        # pack to [NPAIR, H, C, 2, W] -> [NPAIR, H*C, 2*W]
        x3 = np.ascontiguousarray(
            xs.reshape(NPAIR, 2, C, H, W).transpose(0, 3, 2, 1, 4)
        ).reshape(NPAIR, H * C, 2 * W)
        in_maps.append({"x": x3, "w12": w12, "w4x": w4x, "b1": b1})
    res = run_bass_kernel_spmd(nc, in_maps, core_ids=list(range(NCORES)))
    global LAST_RESULT
    LAST_RESULT = res

    out = np.empty((B, CO, HO, WO), np.float32)
    for cr in range(NCORES):
        o2 = res.results[cr]["o"].reshape(NPAIR, CO, HO, 2, WO)
        out[cr * BPC:(cr + 1) * BPC] = (
            o2.transpose(0, 3, 1, 2, 4).reshape(BPC, CO, HO, WO)
        )
    return out


LAST_RESULT = None


# revision 19
# speedup vs baseline: 1.2920x; 1.2920x over previous
"""Trainium2 Bass kernel for the LeNet C3 dense-conv layer.

Computes out = conv2d_valid(x, K, stride 1) + bias where K is the dense
[16, 6, 5, 5] kernel scattered from the sparse per-branch weights
(w3/w4/w6), x is [128, 6, 256, 256] f32, out is [128, 16, 252, 252] f32.

Strategy:
  - Pure data parallelism: 16 images per NeuronCore across 8 cores.
  - The conv is computed as shift-accumulated banded matmuls into PSUM.
    A block covers 6 output rows of FOUR images: the contraction dim
    stacks TWO copies of the 10 input rows (60 partitions each), the
    second copy pre-shifted one column, so each matmul covers two kernel
    columns kx at once: 3 matmuls per image-pair group (kx {0,1}, {2,3},
    {4}) instead of 5. The lhsT is a host-built banded matrix [120, 96]
    whose column m = c_out*6 + r holds K_dense[c_out, c_in, i-r, kx] at
    row i*6 + c_in (+60 for the kx+1 band). Each matmul's moving operand
    carries an image pair (N = 2*256 = 512 = one PSUM bank); the 4-image
    tile feeds two PSUM groups from one input DMA.
  - fp16 operands (~3e-4 rel err; accumulation is fp32 in PSUM). The PE
    here streams 1 column/cycle at 1.2 GHz regardless of dtype, so
    fewer matmul columns == faster; HWDGE rings cost ~9 ns/descriptor,
    so fewer/larger DMAs == faster.
  - Host pre-packs x into x4[q, (h,c), j*256+w] fp16 (2 KB descriptors,
    >=16 outer units per DMA striped over all 16 SDMA engines) and
    un-packs the device output o4[q, c, h, j*252+w] f32 (one 378 KB
    output DMA per block, 4 KB descriptors).
  - PSUM eviction on the vector engine fuses the per-partition bias add.
    The 4 leading pad columns of each input tile keep the 512-wide
    moving operand in-bounds; pads feed only discarded PSUM columns
    (interior images' "pads" are the previous image's tail, same deal).
"""

import numpy as np

# LeNet-5 C3 sparse channel connectivity (from the model definition).
CH3 = np.array([[0, 1, 2], [1, 2, 3], [2, 3, 4], [3, 4, 5], [0, 4, 5], [0, 1, 5]])
CH4 = np.array([[0, 1, 2, 3], [1, 2, 3, 4], [2, 3, 4, 5], [0, 3, 4, 5],
                [0, 1, 4, 5], [0, 1, 2, 5], [0, 1, 3, 4], [1, 2, 4, 5],
                [0, 2, 3, 5]])

B, C, H, W = 128, 6, 256, 256
CO, HO, WO = 16, 252, 252
NCORES = 8
BPC = B // NCORES           # images per core
NQ = BPC // 4               # 4-image groups per core
KH = KW = 5

R = 6                       # output rows per block
HI = R + 4                  # input rows per block
NBLK = HO // R              # 42 blocks per image quad
KK = C * HI                 # contraction rows per kx copy (60)
MM = CO * R                 # psum partitions (96)

_STATE = None  # cached Bass module so repeat kernel() calls skip re-tracing


def _dense_kernel(w3, w4, w6):
    k = np.zeros((CO, C, KH, KW), np.float32)
    k[np.arange(6)[:, None], CH3] = w3
    k[6 + np.arange(9)[:, None], CH4] = w4
    k[15] = w6[0]
    return k


def _band(kd, kx):
    """Banded lhsT [KK, MM] for kernel column kx: row i*6 + c_in,
    column c_out*R + r, value kd[c_out, c_in, i-r, kx]."""
    out = np.zeros((KK, MM), np.float32)
    for ci in range(C):
        for i in range(HI):
            for r in range(R):
                ky = i - r
                if 0 <= ky < KH:
                    out[i * C + ci, np.arange(CO) * R + r] = kd[:, ci, ky, kx]
    return out


def _build_module():
    import concourse.bacc as bacc
    import concourse.mybir as mybir
    from concourse.tile import TileContext

    f32 = mybir.dt.float32
    f16 = mybir.dt.float16

    # Bacc (not Bass): its compile() runs generate_event_semaphores(),
    # which splits multi-wait instructions to satisfy the TRN2 1-wait-
    # per-instruction constraint walrus enforces.
    nc = bacc.Bacc(None)
    # x4[q, (h, c), j*256 + w] = x[4q + j, c, h, w]  (host pre-pack)
    x_d = nc.dram_tensor("x", [NQ, H * C, 4 * W], f16, kind="ExternalInput")
    # w12: [120, 2*96] = [B(0); B(1)] | [B(2); B(3)].  w4x: [60, 96] = B(4).
    w12_d = nc.dram_tensor("w12", [2 * KK, 2 * MM], f16, kind="ExternalInput")
    w4x_d = nc.dram_tensor("w4x", [KK, MM], f16, kind="ExternalInput")
    b1_d = nc.dram_tensor("b1", [MM, 1], f32, kind="ExternalInput")
    # o4[q, c, h, j*252 + w] = out[4q + j, c, h, w]  (host un-packs)
    o_d = nc.dram_tensor("o", [NQ, CO, HO, 4 * WO], f32, kind="ExternalOutput")

    with TileContext(nc) as tc:
        with (
            tc.tile_pool(name="wpool", bufs=1) as wp,
            tc.tile_pool(name="inpool", bufs=8) as ip,
            tc.tile_pool(name="outpool", bufs=6) as op,
            tc.tile_pool(name="pspool", bufs=6, space="PSUM") as pp,
        ):
            w12_t = wp.tile([2 * KK, 2 * MM], f16)
            nc.sync.dma_start(w12_t[:], w12_d[:])
            w4x_t = wp.tile([KK, MM], f16)
            nc.sync.dma_start(w4x_t[:], w4x_d[:])
            b1_t = wp.tile([MM, 1], f32)
            nc.sync.dma_start(b1_t[:], b1_d[:])

            # Prime each constant tile on its consuming engine class so
            # steady-state instructions carry few semaphore waits.
            prime_ps = pp.tile([MM, 192], f32, tag="ps")
            nc.tensor.matmul(prime_ps[:], w12_t[:, 0:MM], w12_t[:, 0:192],
                             start=True, stop=True)
            prime_ps2 = pp.tile([MM, 96], f32, tag="ps")
            nc.tensor.matmul(prime_ps2[:], w4x_t[:, 0:MM], w4x_t[:, 0:96],
                             start=True, stop=True)
            prime_b = op.tile([MM, 1], f32, tag="out")
            nc.vector.tensor_scalar_add(prime_b[:], b1_t[:], 0.0)

            for q in range(NQ):
                for g in range(NBLK):
                    h0 = R * g

                    # Input tile: rows 0..59 hold x cols at t-4 (copy 1),
                    # rows 60..119 the same data at t-3 (copy 2, i.e.
                    # pre-shifted one column for the kx+1 band).
                    it = ip.tile([2 * KK, 4 + 4 * W], f16, tag="in")
                    src = x_d[q, h0 * C:(h0 + HI) * C, :]
                    nc.sync.dma_start(it[0:KK, 4:4 + 4 * W], src)
                    nc.sync.dma_start(it[KK:2 * KK, 3:3 + 4 * W], src)

                    ot = op.tile([MM, 4 * WO], f32, tag="out")
                    for grp in range(2):        # image pairs (0,1), (2,3)
                        base = 512 * grp
                        ps = pp.tile([MM, 2, 256], f32, tag="ps")
                        # q=0 -> kx {0,1}; q=2 -> kx {2,3}; q=4 -> kx 4.
                        nc.tensor.matmul(ps[:, :, :], w12_t[:, 0:MM],
                                         it[:, base:base + 512],
                                         start=True, stop=False)
                        nc.tensor.matmul(ps[:, :, :], w12_t[:, MM:2 * MM],
                                         it[:, base + 2:base + 514],
                                         start=False, stop=False)
                        nc.tensor.matmul(ps[:, :, :], w4x_t[:],
                                         it[0:KK, base + 4:base + 516],
                                         start=False, stop=True)
                        nc.vector.tensor_scalar_add(
                            ot[:, 2 * WO * grp:2 * WO * (grp + 1)]
                            .rearrange("p (j w) -> p j w", j=2),
                            ps[:, :, 4:4 + WO],
                            b1_t[:, 0:1],
                        )
                    nc.scalar.dma_start(o_d[q, :, h0:h0 + R, :], ot[:])
    nc.compile()
    return nc


def _get_module():
    global _STATE
    if _STATE is None:
        _STATE = _build_module()
    return _STATE


def kernel(x, w3, b3, w4, b4, w6, b6):
    from concourse.bass_utils import run_bass_kernel_spmd

    x = np.asarray(x, np.float32)
    kd = _dense_kernel(np.asarray(w3, np.float32), np.asarray(w4, np.float32),
                       np.asarray(w6, np.float32))
    bias = np.concatenate([np.asarray(b3, np.float32),
                           np.asarray(b4, np.float32),
                           np.asarray(b6, np.float32)])

    w12 = np.concatenate([
        np.concatenate([_band(kd, 0), _band(kd, 2)], axis=1),   # rows 0..59
        np.concatenate([_band(kd, 1), _band(kd, 3)], axis=1),   # rows 60..119
    ], axis=0).astype(np.float16)
    w4x = _band(kd, 4).astype(np.float16)
    b1 = np.repeat(bias, R).astype(np.float32).reshape(MM, 1)

    nc = _get_module()
    x16 = x.astype(np.float16)
    in_maps = []
    for cr in range(NCORES):
        xs = x16[cr * BPC:(cr + 1) * BPC]
        # pack to [NQ, H, C, 4, W] -> [NQ, H*C, 4*W]
        x4 = np.ascontiguousarray(
            xs.reshape(NQ, 4, C, H, W).transpose(0, 3, 2, 1, 4)
        ).reshape(NQ, H * C, 4 * W)
        in_maps.append({"x": x4, "w12": w12, "w4x": w4x, "b1": b1})
    res = run_bass_kernel_spmd(nc, in_maps, core_ids=list(range(NCORES)))
    global LAST_RESULT
    LAST_RESULT = res

    out = np.empty((B, CO, HO, WO), np.float32)
    for cr in range(NCORES):
        o4 = res.results[cr]["o"].reshape(NQ, CO, HO, 4, WO)
        out[cr * BPC:(cr + 1) * BPC] = (
            o4.transpose(0, 3, 1, 2, 4).reshape(BPC, CO, HO, WO)
        )
    return out


LAST_RESULT = None
